# revision 24
# baseline (speedup 1.0000x reference)
"""Trainium2 kernel for nn_Attention_local_4088808866313 (sparse windowed attention).

Sharding: data-parallel over batch b (8 cores, one batch element each).
Device per core: full pipeline — x transpose (xbar DMA), depthwise 5x5
conv + folded BN (PE diagonal matmuls), masked dense attention per head
(QK^T with an additive -30 window-mask bias folded into the PSUM
accumulation as a second matmul, exp on ACT, AV with an appended
ones-column in V to produce softmax denominators), normalization and
the output quadrant permutation. Host: BN fold + top-8 window routing
(numpy argpartition) shipped as indices; mask matrices are built
on-device from the indices with match_replace.
"""

import os
import numpy as np
import ml_dtypes

BF = ml_dtypes.bfloat16

B, L, D = 8, 1024, 768
HEADS, DH = 16, 48
H = W = 32
H2 = W2 = 16
P2 = 256
K = 8
PW = 36  # padded image side (32 + 2*2)
EPS = 1e-5
NEG = -30.0  # additive mask bias; exp(-30) ~ 1e-13, negligible vs selected keys

LAST_EXEC_NS = None

# consts blob column layout (bf16 [128, 640])
C_IOTA = 0    # [128, 256] each row = 0..255
C_ID = 256    # [128, 128] identity
C_REVEN = 384  # [128, 128] R_even[k,p] = (k == 16*(p//32) + p%16)
C_RODD = 512   # [128, 128] R_odd[k,p]  = (k == 64 + 16*(p//32) + p%16)


def _emit_device(nc, x_in, w_in, bias_in, idx_in, consts_in, o_out, dbg=None, phase=3):
    from concourse import mybir
    import concourse.bass as bass
    import concourse.tile as tile

    F32 = mybir.dt.float32
    BF16 = mybir.dt.bfloat16
    tc_ctx = tile.TileContext(nc)
    with tc_ctx as tc:
        with (
            tc.tile_pool(name="const", bufs=1) as constp,
            tc.tile_pool(name="chm", bufs=1) as chmp,      # Q/K/V persistent
            tc.tile_pool(name="imgs", bufs=1) as imgp,
            tc.tile_pool(name="xload", bufs=2) as xp,
            tc.tile_pool(name="tmpp", bufs=2) as tmpp,
            tc.tile_pool(name="redp", bufs=2) as redp,
            tc.tile_pool(name="accp", bufs=2) as accp,
            tc.tile_pool(name="maskp", bufs=3) as maskp,
            tc.tile_pool(name="bwtp", bufs=1) as bwtp,
            tc.tile_pool(name="ptp", bufs=8) as ptp,
            tc.tile_pool(name="otp", bufs=2) as otp,
            tc.tile_pool(name="finp", bufs=2) as finp,
            tc.tile_pool(name="pss", bufs=2, space="PSUM") as pssp,
            tc.tile_pool(name="pso", bufs=1, space="PSUM") as psop,
            tc.tile_pool(name="pst", bufs=2, space="PSUM") as pstp,
        ):
            # ---------------- constants / small inputs ----------------
            consts = constp.tile([128, 640], BF16, tag="consts")
            nc.sync.dma_start(consts[:], consts_in[:])
            w_sb = constp.tile([128, 600], BF16, tag="w_sb")
            nc.sync.dma_start(w_sb[:], w_in[:])
            bias_sb = constp.tile([128, 24], F32, tag="bias_sb")
            nc.sync.dma_start(bias_sb[:], bias_in[:])
            idx_sb = constp.tile([128, 256], BF16, tag="idx_sb")
            nc.sync.dma_start(idx_sb[:], idx_in[:])

            iotaM = consts[:, C_IOTA:C_IOTA + 256]
            ident = consts[:, C_ID:C_ID + 128]
            r_even = consts[:, C_REVEN:C_REVEN + 128]
            r_odd = consts[:, C_RODD:C_RODD + 128]

            # ---------------- mask build: bwT[k2] [128 kw, 16h*256 qw] ----------
            bwT = [bwtp.tile([128, 4096], BF16, tag=f"bwt{k2}", name=f"bwt{k2}") for k2 in range(2)]
            for h in range(HEADS if phase >= 2 else 0):
                for i in range(2):
                    a = 2 * h + i
                    m1 = maskp.tile([128, 256], BF16, tag="m1")
                    nc.vector.match_replace(
                        m1[:], idx_sb[:, 8 * a:8 * a + 8], iotaM, -1.0
                    )
                    for k2 in range(2):
                        psB = pstp.tile([128, 128], BF16, tag="pst")
                        nc.tensor.transpose(
                            psB[:], m1[:, 128 * k2:128 * k2 + 128], ident
                        )
                        # fused convert: selected (-1) -> 0, else -> NEG
                        nc.vector.tensor_scalar(
                            bwT[k2][:, h * 256 + 128 * i: h * 256 + 128 * i + 128],
                            psB[:], 0.0, NEG,
                            op0=mybir.AluOpType.is_ge, op1=mybir.AluOpType.mult,
                        )

            if dbg is not None:
                nc.sync.dma_start(dbg["bwt0"][:], bwT[0][:])
                nc.sync.dma_start(dbg["bwt1"][:], bwT[1][:])

            # ---------------- x load: transpose to padded channel-major ---------
            imgs = []
            for b in range(8):
                img = imgp.tile([128, PW * PW], BF16, tag=f"img{b}")
                nc.gpsimd.memset(img[:], 0.0)
                imgs.append(img)
            for b in range(8):
                xt = xp.tile([128, 1024], BF16, tag="xt")
                nc.gpsimd.memset(xt[:], 0.0)
                nc.sync.dma_start_transpose(
                    xt[0:48, :], x_in[:, 96 * b:96 * b + 48]
                )
                nc.sync.dma_start_transpose(
                    xt[64:112, :], x_in[:, 96 * b + 48:96 * b + 96]
                )
                dv = imgs[b][:].rearrange("p (a b) -> p a b", a=PW)[:, 2:34, 2:34]
                sv = xt[:].rearrange("p (a b) -> p a b", a=H)
                nc.vector.tensor_copy(dv, sv)

            # ---------------- conv on DVE: broadcast-mult + reduce ----------
            # out[c, y, x] = sum_dy sum_dx w[c, dy, dx] * xpad[c, y+dy, x+dx]
            # per dy: one TT (w broadcast over pixels) + one reduce over dx
            q_chm, k_chm = [], []
            v_pix = [chmp.tile([128, 1024], BF16, tag=f"vpix{c8}", name=f"vpix{c8}") for c8 in range(8)]
            for b in range(8):
                img3 = imgs[b][:].rearrange("p (a b) -> p a b", a=PW)
                for j in range(3):
                    cw = b * 75 + j * 25
                    acc = accp.tile([128, 1024], F32, tag="acc")
                    for dy in range(5):
                        sl = img3[:, dy:dy + 32, :]
                        in0 = bass.AP(
                            tensor=sl.tensor,
                            offset=sl.offset,
                            ap=[list(p) for p in sl.ap[:2]]
                            + [[1, 32], [1, 5]],
                        )
                        wsl = w_sb[:, cw + 5 * dy:cw + 5 * dy + 5]
                        tmp = tmpp.tile([128, 5120], BF16, tag="tmp")
                        nc.vector.tensor_mul(
                            tmp[:].rearrange("p (y x t) -> p y x t", y=32, x=32),
                            in0,
                            wsl[:, None, None, :].broadcast_to([128, 32, 32, 5]),
                        )
                        if dy == 0:
                            nc.vector.tensor_reduce(
                                acc[:],
                                tmp[:].rearrange("p (y t) -> p y t", t=5),
                                axis=mybir.AxisListType.X,
                                op=mybir.AluOpType.add,
                            )
                        else:
                            red = redp.tile([128, 1024], F32, tag="red")
                            nc.vector.tensor_reduce(
                                red[:],
                                tmp[:].rearrange("p (y t) -> p y t", t=5),
                                axis=mybir.AxisListType.X,
                                op=mybir.AluOpType.add,
                            )
                            nc.vector.tensor_add(acc[:], acc[:], red[:])
                    bcol = b * 3 + j
                    dst = chmp.tile(
                        [128, 1024], BF16,
                        tag=("vchm" if j == 2 else f"chm{j}_{b}"),
                        bufs=(2 if j == 2 else None),
                        name=f"chm{j}_{b}",
                    )
                    nc.scalar.activation(
                        dst[:], acc[:],
                        mybir.ActivationFunctionType.Identity,
                        bias=bias_sb[:, bcol:bcol + 1], scale=1.0,
                    )
                    if j == 0:
                        q_chm.append(dst)
                    elif j == 1:
                        k_chm.append(dst)
                    else:
                        # V: rows 48/112 are all-ones via the conv bias
                        # (zero weights + bias=1) -> softmax denominator
                        for c8 in range(8):
                            psT = pstp.tile([128, 128], BF16, tag="pst")
                            nc.tensor.transpose(
                                psT[:], dst[:, 128 * c8:128 * c8 + 128], ident
                            )
                            nc.vector.tensor_copy(
                                v_pix[c8][:, 128 * b:128 * b + 128], psT[:]
                            )

            if dbg is not None:
                nc.sync.dma_start(dbg["qchm0"][:], q_chm[0][:])
                nc.sync.dma_start(dbg["kchm0"][:], k_chm[0][:])
                nc.sync.dma_start(dbg["vpix0"][:], v_pix[0][:])

            # ---------------- attention, two heads (one ptile) at a time ------
            # out_pix[c8] col layout: pair hp occupies cols 128*hp..128*hp+127
            # (head 2hp at +0..48 incl colsum at +48, head 2hp+1 at +64..112)
            out_pix = [chmp.tile([128, 1024], BF16, tag=f"opix{c8}", name=f"opix{c8}") for c8 in range(8)]
            for hp in range(8 if phase >= 3 else 0):
                psO = psop.tile([128, 1024], F32, tag="pso")
                oT = otp.tile([128, 1024], BF16, tag="ot")
                nc.gpsimd.memset(oT[:], 0.0)
                for slot in range(2):
                    h = 2 * hp + slot
                    p0 = 64 * slot
                    pts = []
                    for c in range(8):
                        cp = c % 4
                        k2 = cp // 2
                        rmat = r_even if (cp % 2 == 0) else r_odd
                        psS = pssp.tile([128, 1024], F32, tag="pss")
                        sl = bwT[k2][:, h * 256:h * 256 + 256]
                        for n0 in range(2):
                            nc.tensor.matmul(
                                psS[:, 512 * n0:512 * n0 + 512],
                                k_chm[hp][p0:p0 + 64, 128 * c:128 * c + 128],
                                q_chm[hp][p0:p0 + 64, 512 * n0:512 * n0 + 512],
                                start=True, stop=False,
                            )
                            # additive window-mask bias: rhs = bwT with the
                            # (qw -> qpx) expansion AP [(rq,16),(jq,0x2),(sq,1)]
                            rhs = bass.AP(
                                tensor=sl.tensor,
                                offset=sl.offset,
                                ap=[list(p) for p in sl.ap[:1]]
                                + [[16, 16], [0, 2], [1, 16]],
                            )
                            nc.tensor.matmul(
                                psS[:, 512 * n0:512 * n0 + 512],
                                rmat, rhs, start=False, stop=True,
                            )
                        pt = ptp.tile([128, 1024], BF16, tag="pt")
                        nc.scalar.activation(
                            pt[:], psS[:], mybir.ActivationFunctionType.Exp,
                            bias=0.0, scale=1.0,
                        )
                        pts.append(pt)
                        if dbg is not None and h == 0 and c == 0:
                            nc.sync.dma_start(dbg["pt00"][:], pt[:])
                    for n0 in range(2):
                        for c in range(8):
                            nc.tensor.matmul(
                                psO[p0:p0 + 49, 512 * n0:512 * n0 + 512],
                                v_pix[c][:, 128 * hp + p0:128 * hp + p0 + 49],
                                pts[c][:, 512 * n0:512 * n0 + 512],
                                start=(c == 0), stop=(c == 7),
                            )
                    nc.vector.tensor_copy(
                        oT[p0:p0 + 49, :], psO[p0:p0 + 49, :]
                    )
                if dbg is not None and hp == 0:
                    nc.sync.dma_start(dbg["ot0"][:], oT[0:49, :])
                for c8 in range(8):
                    psT2 = pstp.tile([128, 128], BF16, tag="pst")
                    nc.tensor.transpose(
                        psT2[:], oT[:, 128 * c8:128 * c8 + 128], ident
                    )
                    nc.vector.tensor_copy(
                        out_pix[c8][:, 128 * hp:128 * hp + 128], psT2[:]
                    )

            # ---------------- normalize + store (host does quadrant perm) ----
            for c8 in range(8 if phase >= 3 else 0):
                rc = finp.tile([128, 16], BF16, tag="rc")
                opv = out_pix[c8][:].rearrange("p (a s) -> p a s", a=8)
                with nc.allow_low_precision(reason="softmax denom recip in bf16; 0.4% rel err ok"):
                    nc.vector.reciprocal(
                        rc[:].rearrange("p (a s) -> p a s", a=8),
                        opv[:, :, 48:113:64],
                    )
                fin = finp.tile([128, 768], BF16, tag="fin")
                nc.vector.tensor_mul(
                    fin[:].rearrange("p (a s c) -> p a s c", a=8, s=2),
                    out_pix[c8][:].rearrange("p (a s q) -> p a s q", a=8, s=2)[:, :, :, 0:48],
                    rc[:].rearrange("p (a s) -> p a s", a=8)[:, :, :, None].broadcast_to([128, 8, 2, 48]),
                )
                nc.sync.dma_start(
                    o_out[128 * c8:128 * c8 + 128, :], fin[:]
                )
    return nc


def _build_program():
    from concourse import bacc, mybir

    nc = bacc.Bacc("TRN2", target_bir_lowering=False)
    F32 = mybir.dt.float32
    BF16 = mybir.dt.bfloat16

    x_in = nc.dram_tensor("x_in", [L, D], BF16, kind="ExternalInput")
    w_in = nc.dram_tensor("w_in", [128, 600], BF16, kind="ExternalInput")
    bias_in = nc.dram_tensor("bias_in", [128, 24], F32, kind="ExternalInput")
    idx_in = nc.dram_tensor("idx_in", [128, 256], BF16, kind="ExternalInput")
    consts_in = nc.dram_tensor("consts_in", [128, 640], BF16, kind="ExternalInput")
    o_out = nc.dram_tensor("o_out", [L, D], BF16, kind="ExternalOutput")

    _emit_device(nc, x_in, w_in, bias_in, idx_in, consts_in, o_out)
    nc.finalize()
    return nc


def _chan_of(b, p):
    if p < 48:
        return 96 * b + p
    if 64 <= p < 112:
        return 96 * b + 48 + (p - 64)
    return -1


def _host_prepare(conv_w, bn_gamma, bn_beta, bn_mean, bn_var):
    inv = bn_gamma / np.sqrt(bn_var + EPS)  # (3, 768)
    w_eff = conv_w[:, :, 0, :, :] * inv[:, :, None, None]  # (3, 768, 5, 5)
    b_eff = bn_beta - bn_mean * inv  # (3, 768)
    scale = float(D) ** -0.5
    w_eff = w_eff.copy()
    b_eff = b_eff.copy()
    w_eff[0] *= scale
    b_eff[0] *= scale

    w600 = np.zeros((128, 600), np.float32)
    bias24 = np.zeros((128, 24), np.float32)
    for b in range(8):
        for p in range(128):
            ch = _chan_of(b, p)
            if ch < 0:
                continue
            for j in range(3):
                w600[p, b * 75 + j * 25: b * 75 + j * 25 + 25] = w_eff[j, ch].reshape(25)
                bias24[p, b * 3 + j] = b_eff[j, ch]
        # V ones-rows (pad rows 48/112 have zero weights): bias 1.0 makes the
        # conv emit constant 1.0 there -> softmax denominator column in AV
        bias24[48, b * 3 + 2] = 1.0
        bias24[112, b * 3 + 2] = 1.0

    consts = np.zeros((128, 640), np.float32)
    consts[:, C_IOTA:C_IOTA + 256] = np.arange(256)[None, :]
    consts[:, C_ID:C_ID + 128] = np.eye(128)
    p = np.arange(128)
    kloc = 16 * (p // 32) + p % 16
    consts[:, C_REVEN:C_REVEN + 128] = (np.arange(128)[:, None] == kloc[None, :])
    consts[:, C_RODD:C_RODD + 128] = (np.arange(128)[:, None] == (64 + kloc)[None, :])
    return w600.astype(BF), bias24, consts.astype(BF)


def _topk_idx(gen_adj):
    # (nb, 16, 256, 256) -> idx_sb (nb, 128, 256) bf16 (set semantics; order free)
    nb = gen_adj.shape[0]
    flat = gen_adj.reshape(nb * HEADS * P2, P2)
    part = np.argpartition(flat, P2 - K, axis=-1)[:, P2 - K:]  # (nb*H*P2, 8)
    idx4 = part.reshape(nb, HEADS, 2, 128, K).transpose(0, 3, 1, 2, 4)
    return np.ascontiguousarray(idx4.reshape(nb, 128, 256)).astype(BF)


def kernel(x, noise, gen_adj, conv_w, bn_gamma, bn_beta, bn_mean, bn_var, sparsity):
    global LAST_EXEC_NS
    from concourse.bass_utils import run_bass_kernel_spmd

    assert int(sparsity) == K
    x = np.asarray(x, np.float32)
    gen_adj = np.asarray(gen_adj, np.float32)
    w600, bias24, consts = _host_prepare(
        np.asarray(conv_w, np.float32),
        np.asarray(bn_gamma, np.float32),
        np.asarray(bn_beta, np.float32),
        np.asarray(bn_mean, np.float32),
        np.asarray(bn_var, np.float32),
    )
    idx_sb = _topk_idx(gen_adj)
    x_bf = x.astype(BF)

    nc = _build_program()
    in_maps = []
    for bb in range(B):
        in_maps.append(
            {
                "x_in": np.ascontiguousarray(x_bf[bb]),
                "w_in": w600,
                "bias_in": bias24,
                "idx_in": np.ascontiguousarray(idx_sb[bb]),
                "consts_in": consts,
            }
        )

    trace = os.environ.get("KERNEL_TRACE", "0") == "1"
    res = run_bass_kernel_spmd(
        nc, in_maps, core_ids=list(range(B)), trace=trace
    )
    if trace:
        LAST_EXEC_NS = res.exec_time_ns
    if os.environ.get("KERNEL_TIME", "0") == "1":
        # second run hits the in-process PJRT executable cache; wall-time it
        import time as _time

        t0 = _time.time()
        res = run_bass_kernel_spmd(
            nc, in_maps, core_ids=list(range(B)), trace=False
        )
        LAST_EXEC_NS = int((_time.time() - t0) * 1e9)

    o = np.stack([np.asarray(r["o_out"], np.float32) for r in res.results])
    # quadrant permutation: out pixel (jq*16+r, iq*16+s) <- device row (iq*16+r, jq*16+s)
    o = o.reshape(B, 2, 16, 2, 16, D).transpose(0, 3, 2, 1, 4, 5)
    return np.ascontiguousarray(o.reshape(B, L, D))


if __name__ == "__main__":
    rng = np.random.default_rng(0)
    inputs = {
        "x": rng.standard_normal((B, L, D), dtype=np.float32),
        "noise": np.zeros((1,), np.float32),
        "gen_adj": rng.standard_normal((B, HEADS, P2, P2), dtype=np.float32),
        "conv_w": (rng.standard_normal((3, D, 1, 5, 5)) * 0.1).astype(np.float32),
        "bn_gamma": (1.0 + 0.1 * rng.standard_normal((3, D))).astype(np.float32),
        "bn_beta": (0.1 * rng.standard_normal((3, D))).astype(np.float32),
        "bn_mean": (0.1 * rng.standard_normal((3, D))).astype(np.float32),
        "bn_var": rng.uniform(0.5, 1.5, (3, D)).astype(np.float32),
        "sparsity": 8,
    }
    out = kernel(**inputs)
    print(out.shape, out.dtype, float(np.abs(out).max()))


# revision 25
# speedup vs baseline: 1.2647x; 1.2647x over previous
"""Trainium2 kernel for nn_Attention_local_4088808866313 (sparse windowed attention).

Sharding: data-parallel over batch b (8 cores, one batch element each).
Device per core: full pipeline — x transpose (xbar DMA), depthwise 5x5
conv + folded BN (PE diagonal matmuls), masked dense attention per head
(QK^T with an additive -30 window-mask bias folded into the PSUM
accumulation as a second matmul, exp on ACT, AV with an appended
ones-column in V to produce softmax denominators), normalization and
the output quadrant permutation. Host: BN fold + top-8 window routing
(numpy argpartition) shipped as indices; mask matrices are built
on-device from the indices with match_replace.
"""

import os
import numpy as np
import ml_dtypes

BF = ml_dtypes.bfloat16

B, L, D = 8, 1024, 768
HEADS, DH = 16, 48
H = W = 32
H2 = W2 = 16
P2 = 256
K = 8
PW = 36  # padded image side (32 + 2*2)
EPS = 1e-5
NEG = -30.0  # additive mask bias; exp(-30) ~ 1e-13, negligible vs selected keys

LAST_EXEC_NS = None

# consts blob column layout (bf16 [128, 640])
C_IOTA = 0    # [128, 256] each row = 0..255
C_ID = 256    # [128, 128] identity
C_REVEN = 384  # [128, 128] R_even[k,p] = (k == 16*(p//32) + p%16)
C_RODD = 512   # [128, 128] R_odd[k,p]  = (k == 64 + 16*(p//32) + p%16)


def _emit_device(nc, x_in, blob_in, bias_in, o_out, dbg=None, phase=3):
    from concourse import mybir
    import concourse.bass as bass
    import concourse.tile as tile

    F32 = mybir.dt.float32
    BF16 = mybir.dt.bfloat16
    tc_ctx = tile.TileContext(nc)
    with tc_ctx as tc:
        with (
            tc.tile_pool(name="const", bufs=1) as constp,
            tc.tile_pool(name="chm", bufs=1) as chmp,      # Q/K/V persistent
            tc.tile_pool(name="imgs", bufs=1) as imgp,
            tc.tile_pool(name="xload", bufs=2) as xp,
            tc.tile_pool(name="tmpp", bufs=2) as tmpp,
            tc.tile_pool(name="redp", bufs=2) as redp,
            tc.tile_pool(name="accp", bufs=2) as accp,
            tc.tile_pool(name="maskp", bufs=3) as maskp,
            tc.tile_pool(name="bwtp", bufs=1) as bwtp,
            tc.tile_pool(name="ptp", bufs=8) as ptp,
            tc.tile_pool(name="otp", bufs=2) as otp,
            tc.tile_pool(name="finp", bufs=2) as finp,
            tc.tile_pool(name="pss", bufs=2, space="PSUM") as pssp,
            tc.tile_pool(name="pso", bufs=1, space="PSUM") as psop,
            tc.tile_pool(name="pst", bufs=2, space="PSUM") as pstp,
        ):
            # ---------------- constants / small inputs (one blob) -------------
            blob = constp.tile([128, 1496], BF16, tag="blob")
            nc.sync.dma_start(blob[:], blob_in[:])
            bias_sb = constp.tile([128, 24], F32, tag="bias_sb")
            nc.sync.dma_start(bias_sb[:], bias_in[:])

            w_sb = blob[:, 0:600]
            idx_sb = blob[:, 600:856]
            consts = blob[:, 856:1496]
            iotaM = consts[:, C_IOTA:C_IOTA + 256]
            ident = consts[:, C_ID:C_ID + 128]
            r_even = consts[:, C_REVEN:C_REVEN + 128]
            r_odd = consts[:, C_RODD:C_RODD + 128]

            # ---------------- mask build: bwT[k2] [128 kw, 16h*256 qw] ----------
            bwT = [bwtp.tile([128, 4096], BF16, tag=f"bwt{k2}", name=f"bwt{k2}") for k2 in range(2)]
            for h in range(HEADS if phase >= 2 else 0):
                for i in range(2):
                    a = 2 * h + i
                    m1 = maskp.tile([128, 256], BF16, tag="m1")
                    nc.vector.match_replace(
                        m1[:], idx_sb[:, 8 * a:8 * a + 8], iotaM, -1.0
                    )
                    for k2 in range(2):
                        psB = pstp.tile([128, 128], BF16, tag="pst")
                        nc.tensor.transpose(
                            psB[:], m1[:, 128 * k2:128 * k2 + 128], ident
                        )
                        # fused convert: selected (-1) -> 0, else -> NEG
                        nc.vector.tensor_scalar(
                            bwT[k2][:, h * 256 + 128 * i: h * 256 + 128 * i + 128],
                            psB[:], 0.0, NEG,
                            op0=mybir.AluOpType.is_ge, op1=mybir.AluOpType.mult,
                        )

            if dbg is not None:
                nc.sync.dma_start(dbg["bwt0"][:], bwT[0][:])
                nc.sync.dma_start(dbg["bwt1"][:], bwT[1][:])

            # ---------------- x load: transpose to padded channel-major ---------
            imgs = []
            for b in range(8):
                img = imgp.tile([128, PW * PW], BF16, tag=f"img{b}")
                nc.gpsimd.memset(img[:], 0.0)
                imgs.append(img)
            for b in range(8):
                xt = xp.tile([128, 1024], BF16, tag="xt")
                nc.gpsimd.memset(xt[:], 0.0)
                nc.sync.dma_start_transpose(
                    xt[0:48, :], x_in[:, 96 * b:96 * b + 48]
                )
                nc.sync.dma_start_transpose(
                    xt[64:112, :], x_in[:, 96 * b + 48:96 * b + 96]
                )
                dv = imgs[b][:].rearrange("p (a b) -> p a b", a=PW)[:, 2:34, 2:34]
                sv = xt[:].rearrange("p (a b) -> p a b", a=H)
                nc.vector.tensor_copy(dv, sv)

            # ---------------- conv on DVE: broadcast-mult + reduce ----------
            # out[c, y, x] = sum_dy sum_dx w[c, dy, dx] * xpad[c, y+dy, x+dx]
            # per dy: one TT (w broadcast over pixels) + one reduce over dx
            q_chm, k_chm = [], []
            v_pix = [chmp.tile([128, 1024], BF16, tag=f"vpix{c8}", name=f"vpix{c8}") for c8 in range(8)]
            for b in range(8):
                img3 = imgs[b][:].rearrange("p (a b) -> p a b", a=PW)
                for j in range(3):
                    cw = b * 75 + j * 25
                    acc = accp.tile([128, 1024], F32, tag="acc")
                    for dy in range(5):
                        sl = img3[:, dy:dy + 32, :]
                        in0 = bass.AP(
                            tensor=sl.tensor,
                            offset=sl.offset,
                            ap=[list(p) for p in sl.ap[:2]]
                            + [[1, 32], [1, 5]],
                        )
                        wsl = w_sb[:, cw + 5 * dy:cw + 5 * dy + 5]
                        tmp = tmpp.tile([128, 5120], BF16, tag="tmp")
                        nc.vector.tensor_mul(
                            tmp[:].rearrange("p (y x t) -> p y x t", y=32, x=32),
                            in0,
                            wsl[:, None, None, :].broadcast_to([128, 32, 32, 5]),
                        )
                        if dy == 0:
                            nc.vector.tensor_reduce(
                                acc[:],
                                tmp[:].rearrange("p (y t) -> p y t", t=5),
                                axis=mybir.AxisListType.X,
                                op=mybir.AluOpType.add,
                            )
                        else:
                            red = redp.tile([128, 1024], F32, tag="red")
                            nc.vector.tensor_reduce(
                                red[:],
                                tmp[:].rearrange("p (y t) -> p y t", t=5),
                                axis=mybir.AxisListType.X,
                                op=mybir.AluOpType.add,
                            )
                            nc.vector.tensor_add(acc[:], acc[:], red[:])
                    bcol = b * 3 + j
                    dst = chmp.tile(
                        [128, 1024], BF16,
                        tag=("vchm" if j == 2 else f"chm{j}_{b}"),
                        bufs=(2 if j == 2 else None),
                        name=f"chm{j}_{b}",
                    )
                    nc.scalar.activation(
                        dst[:], acc[:],
                        mybir.ActivationFunctionType.Identity,
                        bias=bias_sb[:, bcol:bcol + 1], scale=1.0,
                    )
                    if j == 0:
                        q_chm.append(dst)
                    elif j == 1:
                        k_chm.append(dst)
                    else:
                        # V: rows 48/112 are all-ones via the conv bias
                        # (zero weights + bias=1) -> softmax denominator
                        for c8 in range(8):
                            psT = pstp.tile([128, 128], BF16, tag="pst")
                            nc.tensor.transpose(
                                psT[:], dst[:, 128 * c8:128 * c8 + 128], ident
                            )
                            nc.vector.tensor_copy(
                                v_pix[c8][:, 128 * b:128 * b + 128], psT[:]
                            )

            if dbg is not None:
                nc.sync.dma_start(dbg["qchm0"][:], q_chm[0][:])
                nc.sync.dma_start(dbg["kchm0"][:], k_chm[0][:])
                nc.sync.dma_start(dbg["vpix0"][:], v_pix[0][:])

            # ---------------- attention, two heads (one ptile) at a time ------
            # out_pix[c8] col layout: pair hp occupies cols 128*hp..128*hp+127
            # (head 2hp at +0..48 incl colsum at +48, head 2hp+1 at +64..112)
            out_pix = [chmp.tile([128, 1024], BF16, tag=f"opix{c8}", name=f"opix{c8}") for c8 in range(8)]
            for hp in range(8 if phase >= 3 else 0):
                psO = psop.tile([128, 1024], F32, tag="pso")
                oT = otp.tile([128, 1024], BF16, tag="ot")
                nc.gpsimd.memset(oT[:], 0.0)
                for slot in range(2):
                    h = 2 * hp + slot
                    p0 = 64 * slot
                    pts = []
                    for c in range(8):
                        cp = c % 4
                        k2 = cp // 2
                        rmat = r_even if (cp % 2 == 0) else r_odd
                        psS = pssp.tile([128, 1024], F32, tag="pss")
                        sl = bwT[k2][:, h * 256:h * 256 + 256]
                        for n0 in range(2):
                            nc.tensor.matmul(
                                psS[:, 512 * n0:512 * n0 + 512],
                                k_chm[hp][p0:p0 + 64, 128 * c:128 * c + 128],
                                q_chm[hp][p0:p0 + 64, 512 * n0:512 * n0 + 512],
                                start=True, stop=False,
                            )
                            # additive window-mask bias: rhs = bwT with the
                            # (qw -> qpx) expansion AP [(rq,16),(jq,0x2),(sq,1)]
                            rhs = bass.AP(
                                tensor=sl.tensor,
                                offset=sl.offset,
                                ap=[list(p) for p in sl.ap[:1]]
                                + [[16, 16], [0, 2], [1, 16]],
                            )
                            nc.tensor.matmul(
                                psS[:, 512 * n0:512 * n0 + 512],
                                rmat, rhs, start=False, stop=True,
                            )
                        pt = ptp.tile([128, 1024], BF16, tag="pt")
                        nc.scalar.activation(
                            pt[:], psS[:], mybir.ActivationFunctionType.Exp,
                            bias=0.0, scale=1.0,
                        )
                        pts.append(pt)
                        if dbg is not None and h == 0 and c == 0:
                            nc.sync.dma_start(dbg["pt00"][:], pt[:])
                    for n0 in range(2):
                        for c in range(8):
                            nc.tensor.matmul(
                                psO[p0:p0 + 49, 512 * n0:512 * n0 + 512],
                                v_pix[c][:, 128 * hp + p0:128 * hp + p0 + 49],
                                pts[c][:, 512 * n0:512 * n0 + 512],
                                start=(c == 0), stop=(c == 7),
                            )
                    nc.vector.tensor_copy(
                        oT[p0:p0 + 49, :], psO[p0:p0 + 49, :]
                    )
                if dbg is not None and hp == 0:
                    nc.sync.dma_start(dbg["ot0"][:], oT[0:49, :])
                for c8 in range(8):
                    psT2 = pstp.tile([128, 128], BF16, tag="pst")
                    nc.tensor.transpose(
                        psT2[:], oT[:, 128 * c8:128 * c8 + 128], ident
                    )
                    nc.vector.tensor_copy(
                        out_pix[c8][:, 128 * hp:128 * hp + 128], psT2[:]
                    )

            # ---------------- normalize + store (host does quadrant perm) ----
            for c8 in range(8 if phase >= 3 else 0):
                rc = finp.tile([128, 16], BF16, tag="rc")
                opv = out_pix[c8][:].rearrange("p (a s) -> p a s", a=8)
                with nc.allow_low_precision(reason="softmax denom recip in bf16; 0.4% rel err ok"):
                    nc.vector.reciprocal(
                        rc[:].rearrange("p (a s) -> p a s", a=8),
                        opv[:, :, 48:113:64],
                    )
                fin = finp.tile([128, 768], BF16, tag="fin")
                nc.vector.tensor_mul(
                    fin[:].rearrange("p (a s c) -> p a s c", a=8, s=2),
                    out_pix[c8][:].rearrange("p (a s q) -> p a s q", a=8, s=2)[:, :, :, 0:48],
                    rc[:].rearrange("p (a s) -> p a s", a=8)[:, :, :, None].broadcast_to([128, 8, 2, 48]),
                )
                nc.sync.dma_start(
                    o_out[128 * c8:128 * c8 + 128, :], fin[:]
                )
    return nc


def _build_program():
    from concourse import bacc, mybir

    nc = bacc.Bacc("TRN2", target_bir_lowering=False)
    F32 = mybir.dt.float32
    BF16 = mybir.dt.bfloat16

    x_in = nc.dram_tensor("x_in", [L, D], BF16, kind="ExternalInput")
    blob_in = nc.dram_tensor("blob_in", [128, 1496], BF16, kind="ExternalInput")
    bias_in = nc.dram_tensor("bias_in", [128, 24], F32, kind="ExternalInput")
    o_out = nc.dram_tensor("o_out", [L, D], BF16, kind="ExternalOutput")

    _emit_device(nc, x_in, blob_in, bias_in, o_out)
    nc.finalize()
    return nc


def _chan_of(b, p):
    if p < 48:
        return 96 * b + p
    if 64 <= p < 112:
        return 96 * b + 48 + (p - 64)
    return -1


def _host_prepare(conv_w, bn_gamma, bn_beta, bn_mean, bn_var):
    inv = bn_gamma / np.sqrt(bn_var + EPS)  # (3, 768)
    w_eff = conv_w[:, :, 0, :, :] * inv[:, :, None, None]  # (3, 768, 5, 5)
    b_eff = bn_beta - bn_mean * inv  # (3, 768)
    scale = float(D) ** -0.5
    w_eff = w_eff.copy()
    b_eff = b_eff.copy()
    w_eff[0] *= scale
    b_eff[0] *= scale

    w600 = np.zeros((128, 600), np.float32)
    bias24 = np.zeros((128, 24), np.float32)
    for b in range(8):
        for p in range(128):
            ch = _chan_of(b, p)
            if ch < 0:
                continue
            for j in range(3):
                w600[p, b * 75 + j * 25: b * 75 + j * 25 + 25] = w_eff[j, ch].reshape(25)
                bias24[p, b * 3 + j] = b_eff[j, ch]
        # V ones-rows (pad rows 48/112 have zero weights): bias 1.0 makes the
        # conv emit constant 1.0 there -> softmax denominator column in AV
        bias24[48, b * 3 + 2] = 1.0
        bias24[112, b * 3 + 2] = 1.0

    consts = np.zeros((128, 640), np.float32)
    consts[:, C_IOTA:C_IOTA + 256] = np.arange(256)[None, :]
    consts[:, C_ID:C_ID + 128] = np.eye(128)
    p = np.arange(128)
    kloc = 16 * (p // 32) + p % 16
    consts[:, C_REVEN:C_REVEN + 128] = (np.arange(128)[:, None] == kloc[None, :])
    consts[:, C_RODD:C_RODD + 128] = (np.arange(128)[:, None] == (64 + kloc)[None, :])
    return w600.astype(BF), bias24, consts.astype(BF)


def _topk_idx(gen_adj):
    # (nb, 16, 256, 256) -> idx_sb (nb, 128, 256) bf16 (set semantics; order free)
    nb = gen_adj.shape[0]
    flat = gen_adj.reshape(nb * HEADS * P2, P2)
    part = np.argpartition(flat, P2 - K, axis=-1)[:, P2 - K:]  # (nb*H*P2, 8)
    idx4 = part.reshape(nb, HEADS, 2, 128, K).transpose(0, 3, 1, 2, 4)
    return np.ascontiguousarray(idx4.reshape(nb, 128, 256)).astype(BF)


def kernel(x, noise, gen_adj, conv_w, bn_gamma, bn_beta, bn_mean, bn_var, sparsity):
    global LAST_EXEC_NS
    from concourse.bass_utils import run_bass_kernel_spmd

    assert int(sparsity) == K
    x = np.asarray(x, np.float32)
    gen_adj = np.asarray(gen_adj, np.float32)
    w600, bias24, consts = _host_prepare(
        np.asarray(conv_w, np.float32),
        np.asarray(bn_gamma, np.float32),
        np.asarray(bn_beta, np.float32),
        np.asarray(bn_mean, np.float32),
        np.asarray(bn_var, np.float32),
    )
    idx_sb = _topk_idx(gen_adj)
    x_bf = x.astype(BF)

    nc = _build_program()
    in_maps = []
    for bb in range(B):
        blob = np.concatenate([w600, idx_sb[bb], consts], axis=1)
        in_maps.append(
            {
                "x_in": np.ascontiguousarray(x_bf[bb]),
                "blob_in": np.ascontiguousarray(blob),
                "bias_in": bias24,
            }
        )

    trace = os.environ.get("KERNEL_TRACE", "0") == "1"
    res = run_bass_kernel_spmd(
        nc, in_maps, core_ids=list(range(B)), trace=trace
    )
    if trace:
        LAST_EXEC_NS = res.exec_time_ns
    if os.environ.get("KERNEL_TIME", "0") == "1":
        # second run hits the in-process PJRT executable cache; wall-time it
        import time as _time

        t0 = _time.time()
        res = run_bass_kernel_spmd(
            nc, in_maps, core_ids=list(range(B)), trace=False
        )
        LAST_EXEC_NS = int((_time.time() - t0) * 1e9)

    o = np.stack([np.asarray(r["o_out"], np.float32) for r in res.results])
    # quadrant permutation: out pixel (jq*16+r, iq*16+s) <- device row (iq*16+r, jq*16+s)
    o = o.reshape(B, 2, 16, 2, 16, D).transpose(0, 3, 2, 1, 4, 5)
    return np.ascontiguousarray(o.reshape(B, L, D))


if __name__ == "__main__":
    rng = np.random.default_rng(0)
    inputs = {
        "x": rng.standard_normal((B, L, D), dtype=np.float32),
        "noise": np.zeros((1,), np.float32),
        "gen_adj": rng.standard_normal((B, HEADS, P2, P2), dtype=np.float32),
        "conv_w": (rng.standard_normal((3, D, 1, 5, 5)) * 0.1).astype(np.float32),
        "bn_gamma": (1.0 + 0.1 * rng.standard_normal((3, D))).astype(np.float32),
        "bn_beta": (0.1 * rng.standard_normal((3, D))).astype(np.float32),
        "bn_mean": (0.1 * rng.standard_normal((3, D))).astype(np.float32),
        "bn_var": rng.uniform(0.5, 1.5, (3, D)).astype(np.float32),
        "sparsity": 8,
    }
    out = kernel(**inputs)
    print(out.shape, out.dtype, float(np.abs(out).max()))
